# revision 3
# baseline (speedup 1.0000x reference)
"""Dilated attention (LongNet-style) Trainium2 kernel, v2.

Problem: query/key/value (2, 8192, 12, 64) f32. Three dilation groups
(segment lengths 2048/4096/8192, dilation 1/2/4, head slices 0:4/4:8/8:12).
Each group's gather produces independent dense attention over 2048-position
dilated segments; outputs are normalized per (batch, head, channel) by the
sum over all segment positions, and divided by num_groups.

Sharding: 8 cores = 2 batches x 4 "head columns". Core c owns batch c//4 and
heads {j, 4+j, 8+j} where j = c%4 -- exactly 7 dense 2048x2048x64 attention
units per core (4 + 2 + 1 segments), perfectly balanced, with all segments of
any (batch, head) on one core so normalization needs no cross-core traffic.

Precision strategy ("rowsum-corrected fp16 attention"): the reference's
x / x.sum(axis=(1,2)) normalization divides by a nearly-cancelling sum D,
which amplifies per-element noise ~300x. But the amplification flows ONLY
through D: per-element noise in x itself is unamplified. So instead of
computing everything at ~fp32 (the 5-stream fp16 hi/lo baseline), compute
the attention with plain fp16 P and fp16 V (noise ~3e-4, unamplified) and
repair D on the host using two tiny f32-grade per-k exports:
  accS[k,unit] = sum_q P32(k,q)   (ACT exp accum_out, f32 internal)
  accP[k,unit] = sum_q p16(k,q)   (DVE tensor_reduce of the fp16 P)
Host correction (per dilation group, in f64):
  D = sum_pos x  +  sum_chunks rbar_c * sum_k [32v*(accS-accP) + wres*accP]
with rbar_c = mean_q in chunk(1/den_q). The 1/den within-chunk variation is
~3%, so the residual amplified noise is ~3e-2 * 3e-4 * 300 ~ 1e-3.
Scores DO feed the amplified path (they perturb the attention map itself),
so they keep a high-precision path: fp16 main matmul with [kh;kl] K=128
stacking (k at ~fp32, q at fp16) plus an fp8-e4m3 DoubleRow correction for
ql = q - fp16(q) (two-level fp8 ql in the two DR slots, shared fp8 k
weights).  Numpy-simulated end-to-end rel err: 1.2e-3 (gate 2e-2).

Device kernel per (chunk, k-block) unit (28 q-chunks of 512 x 16 k-blocks):
  S = khl_blk.T @ qhh  (fp16, 512 cyc)  + DR fp8 corr (256 cyc), PSUM f32
  p16 = fp16(64*exp(S*ESC)) via ACT, accum_out -> accS   (1 pass)
  accP via DVE tensor_reduce (1 instr per 3-unit round)
  O[65, 512] += w16_blk.T @ p16  (fp16, 512 cyc; row 64 = denominator)
PE: 2.5 streams/unit = 1280 cyc vs baseline's 5 = 2560.
"""

import os
import sys

if "/opt/trn_rl_repo" not in sys.path:
    sys.path.insert(0, "/opt/trn_rl_repo")
if "jax" not in sys.modules:
    os.environ.setdefault("JAX_PLATFORMS", "axon")

import math

import ml_dtypes
import numpy as np

import concourse.bass as bass  # noqa: F401
import concourse.mybir as mybir
import concourse.tile as tile
from concourse import bacc
from concourse.bass_utils import run_bass_kernel_spmd

F32 = mybir.dt.float32
F16 = mybir.dt.float16
F8 = mybir.dt.float8e4
NP_F8 = ml_dtypes.float8_e4m3

B, N, H, D = 2, 8192, 12, 64
NSEG = 7           # segments per core
SEG = 2048         # dilated segment length
NCHUNK = NSEG * 4  # 512-wide q chunks per core
NKB = 16           # 128-row k blocks per segment
NUNIT = NCHUNK * NKB
RW = 3             # units per round (3 PSUM banks per score tile)
QSC = np.float32(256.0)               # fp16 pre-scale for Q/K
VSC = np.float32(32.0)                # fp16 pre-scale for V (and ones col)
ESC = float(0.125 / (256.0 * 256.0))  # exp scale: 1/sqrt(64) + descale
PBIAS = float(math.log(64.0))         # exp bias: P in [0.3, 16K], fp16-safe

_CACHE = {}
LAST_RESULT = {}


def _build_nc():
    nc = bacc.Bacc("TRN2", target_bir_lowering=False, debug=False,
                   enable_asserts=False, num_devices=8)
    qhh = nc.dram_tensor("qhh", [128, NSEG * SEG], F16, kind="ExternalInput")
    khl = nc.dram_tensor("khl", [128, NSEG * SEG], F16, kind="ExternalInput")
    qc8 = nc.dram_tensor("qc8", [64, 2, NSEG * SEG], F8, kind="ExternalInput")
    kc8 = nc.dram_tensor("kc8", [64, 2, NSEG * SEG], F8, kind="ExternalInput")
    w16 = nc.dram_tensor("w16", [128, NSEG * NKB * 65], F16,
                         kind="ExternalInput")
    out = nc.dram_tensor("out", [65, NCHUNK * 512], F32, kind="ExternalOutput")
    acs = nc.dram_tensor("acs", [128, NUNIT], F32, kind="ExternalOutput")
    acp = nc.dram_tensor("acp", [128, NUNIT], F32, kind="ExternalOutput")
    qhh_ap, khl_ap, qc8_ap, kc8_ap, w16_ap, out_ap, acs_ap, acp_ap = (
        qhh.ap(), khl.ap(), qc8.ap(), kc8.ap(), w16.ap(), out.ap(),
        acs.ap(), acp.ap())

    with tile.TileContext(nc) as tc:
        with (
            tc.tile_pool(name="inp", bufs=1) as inp,
            tc.tile_pool(name="pt", bufs=4) as ptp,
            tc.tile_pool(name="osb", bufs=3) as osbp,
            tc.tile_pool(name="score", bufs=2, space="PSUM") as scp,
            tc.tile_pool(name="ot", bufs=2, space="PSUM") as otp,
        ):
            bias_t = inp.tile([128, 1], F32, tag="bias", name="bias_t")
            nc.vector.memset(bias_t[:, :], PBIAS)
            acs_sb = inp.tile([128, NUNIT], F32, tag="acs", name="acs_sb")
            acp_sb = inp.tile([128, NUNIT], F32, tag="acp", name="acp_sb")

            # Warm-up prologue: runs while the input DMAs land. ~32 dummy
            # matmuls keep the PE busy >3.4us so the HAM clock-gate opens
            # before the real rounds, and one dummy exp pulls in the ACT
            # table load (~2.7us) that would otherwise stall round 0.
            wsrc = inp.tile([128, 128], F16, tag="wsrc", name="wsrc")
            wjunk = inp.tile([128, 512], F16, tag="wjunk", name="wjunk")
            nc.vector.memset(wsrc[:, :], 0.01)
            nc.vector.memset(wjunk[:, :], 0.01)
            warm = scp.tile([128, 512 * RW], F32, tag="score", name="warm")
            for i in range(32):
                nc.tensor.matmul(warm[:, (i % 3) * 512:(i % 3 + 1) * 512],
                                 wsrc[:, :], wjunk[:, :],
                                 start=(i < 3), stop=(i >= 29))
            wp = ptp.tile([128, 512 * RW], F16, tag="p16", name="warmp")
            nc.scalar.activation(
                wp[:, :512], warm[:, :512],
                mybir.ActivationFunctionType.Exp, scale=ESC, bias=bias_t[:, :])

            qh_sb, k_sb, qc_sb, kc_sb, w_sb = [], [], [], [], []
            for s in range(NSEG):
                qh = inp.tile([128, SEG], F16, tag=f"qh{s}", name=f"qh{s}")
                kk = inp.tile([128, SEG], F16, tag=f"k{s}", name=f"k{s}")
                qc = inp.tile([64, 2, SEG], F8, tag=f"qc{s}", name=f"qc{s}")
                kc = inp.tile([64, 2, SEG], F8, tag=f"kc{s}", name=f"kc{s}")
                wv = inp.tile([128, NKB * 65], F16, tag=f"wv{s}",
                              name=f"wv{s}")
                # split the first segment's Q/K transfers across DMA queues so
                # round 0 isn't gated on a single ~512KB queue transfer
                nsl_dma = 4 if s == 0 else 1
                for t, ap_ in ((qh, qhh_ap), (kk, khl_ap)):
                    step = SEG // nsl_dma
                    for z in range(nsl_dma):
                        lo = z * step
                        nc.sync.dma_start(
                            t[:, lo:lo + step],
                            ap_[:, s * SEG + lo:s * SEG + lo + step])
                ssl = slice(s * SEG, (s + 1) * SEG)
                nc.sync.dma_start(qc[:, :, :], qc8_ap[:, :, ssl])
                nc.sync.dma_start(kc[:, :, :], kc8_ap[:, :, ssl])
                nc.sync.dma_start(
                    wv[:, :], w16_ap[:, s * NKB * 65:(s + 1) * NKB * 65])
                qh_sb.append(qh)
                k_sb.append(kk)
                qc_sb.append(qc)
                kc_sb.append(kc)
                w_sb.append(wv)

            ot_tiles = {}
            pend1, pend2 = [], []  # PV work lagged by 1 and 2 rounds

            def flush(items):
                for p16ref, i, u in items:
                    cid, kb = divmod(u, NKB)
                    s = cid // 4
                    if kb == 0:
                        ot_tiles[cid] = otp.tile([65, 512], F32, tag="ot",
                                                 name=f"ot{cid}")
                    vsl = slice(kb * 65, (kb + 1) * 65)
                    psl = slice(i * 512, (i + 1) * 512)
                    nc.tensor.matmul(ot_tiles[cid][:, :], w_sb[s][:, vsl],
                                     p16ref[:, psl],
                                     start=(kb == 0), stop=(kb == NKB - 1))
                    if kb == NKB - 1:
                        o_sb = osbp.tile([65, 512], F32, tag="osb",
                                         name=f"osb{cid}")
                        # ACT does this copy: DVE is near its budget with the
                        # per-round reduce; ACT has ~300ns/round of slack.
                        nc.scalar.copy(o_sb[:, :], ot_tiles[cid][:, :])
                        nc.sync.dma_start(
                            out_ap[:, cid * 512:(cid + 1) * 512], o_sb[:, :])

            for r in range((NUNIT + RW - 1) // RW):
                units = range(r * RW, min((r + 1) * RW, NUNIT))
                nu = len(units)
                score = scp.tile([128, 512 * RW], F32, tag="score",
                                 name=f"score{r}")
                for i, u in enumerate(units):
                    cid, kb = divmod(u, NKB)
                    s, c = divmod(cid, 4)
                    osl = slice(i * 512, (i + 1) * 512)
                    csl = slice(c * 512, (c + 1) * 512)
                    ksl = slice(kb * 128, (kb + 1) * 128)
                    nc.tensor.matmul(score[:, osl], k_sb[s][:, ksl],
                                     qh_sb[s][:, csl], start=True, stop=False)
                    nc.tensor.matmul(
                        score[:, osl], kc_sb[s][:, :, ksl],
                        qc_sb[s][:, :, csl], start=False, stop=True,
                        perf_mode=mybir.MatmulPerfMode.DoubleRow)
                p16 = ptp.tile([128, 512 * RW], F16, tag="p16", name=f"p16_{r}")
                for i, u in enumerate(units):
                    isl = slice(i * 512, (i + 1) * 512)
                    nc.scalar.activation(
                        p16[:, isl], score[:, isl],
                        mybir.ActivationFunctionType.Exp, scale=ESC,
                        bias=bias_t[:, :], accum_out=acs_sb[:, u:u + 1])
                u0 = r * RW
                nc.vector.tensor_reduce(
                    acp_sb[:, u0:u0 + nu],
                    p16[:, :512 * nu].rearrange("p (u q) -> p u q", u=nu),
                    axis=mybir.AxisListType.X, op=mybir.AluOpType.add)
                if r < 2:
                    # startup filler: the first PV work arrives only after the
                    # round-0 scores->exp chain; keep the PE streaming through
                    # the pipe-fill with dummies aimed at an OT-pool slot
                    # (idle until round 2).
                    fill = otp.tile([128, 512], F32, tag="ot", name=f"fill{r}")
                    for z in range(5):
                        nc.tensor.matmul(fill[:, :], wsrc[:, :], wjunk[:, :],
                                         start=(z == 0), stop=(z == 4))
                flush(pend2)
                pend2 = pend1
                pend1 = [(p16, i, u) for i, u in enumerate(units)]
            flush(pend2)
            flush(pend1)
            nc.sync.dma_start(acs_ap[:, :], acs_sb[:, :])
            nc.sync.dma_start(acp_ap[:, :], acp_sb[:, :])

    nc.compile()
    return nc


def _gather_segs(query, key, value, core):
    b, j = divmod(core, 4)
    segs = []
    for arr in (query, key, value):
        h0 = arr[b, :, j, :].reshape(4, SEG, D)
        h1 = arr[b, :, 4 + j, :].reshape(2, 4096, D)[:, 1::2, :]
        h2 = arr[b, 2::4, 8 + j, :][None]
        segs.append(np.concatenate([h0, h1, h2], axis=0))  # [7, 2048, 64]
    return segs


def _prep_core(query, key, value, core):
    qs, ks, vs = _gather_segs(query, key, value, core)
    # [64, NSEG*SEG] with col = s*SEG + p
    qt = (qs * QSC).transpose(2, 0, 1).reshape(D, NSEG * SEG)
    kt = (ks * QSC).transpose(2, 0, 1).reshape(D, NSEG * SEG)
    qh = qt.astype(np.float16)
    kh = kt.astype(np.float16)
    kl = (kt - kh).astype(np.float16)
    ql = qt - qh.astype(np.float32)
    ql8hi = (8.0 * ql).astype(NP_F8)
    ql8lo = (8.0 * ql - ql8hi.astype(np.float32)).astype(NP_F8)
    qc8 = np.stack([ql8hi, ql8lo], axis=1)          # [64, 2, 14336]
    k8 = (kt / 8.0).astype(NP_F8)
    kc8 = np.stack([k8, k8], axis=1)                # [64, 2, 14336]
    vv = np.concatenate(
        [vs * VSC, np.full((NSEG, SEG, 1), float(VSC), np.float32)],
        axis=2)  # [7, 2048, 65]
    w16 = (vv.reshape(NSEG, NKB, 128, 65).transpose(2, 0, 1, 3)
           .reshape(128, -1).astype(np.float16))
    return {
        "qhh": np.ascontiguousarray(np.concatenate([qh, qh], axis=0)),
        "khl": np.ascontiguousarray(np.concatenate([kh, kl], axis=0)),
        "qc8": np.ascontiguousarray(qc8),
        "kc8": np.ascontiguousarray(kc8),
        "w16": np.ascontiguousarray(w16),
    }


def _unshard(results, query, key, value, dtype):
    full = np.zeros((B, N, H, D), dtype)
    groups = [(0, 4), (4, 6), (6, 7)]
    for core in range(8):
        b, j = divmod(core, 4)
        _, _, vs = _gather_segs(query, key, value, core)
        o = results[core]["out"].astype(np.float64)
        num, den = o[:64], o[64]                     # [64, 14336], [14336]
        acs = results[core]["acs"].astype(np.float64)  # [128, 448]
        acp = results[core]["acp"].astype(np.float64)
        x = num / den[None, :]
        for g0, g1 in groups:
            gcols = slice(g0 * SEG, g1 * SEG)
            Dv = x[:, gcols].sum(axis=1)             # [64]
            C = np.zeros(64)
            for s in range(g0, g1):
                v32 = (VSC * vs[s]).astype(np.float64)       # [2048, 64]
                wres = v32 - v32.astype(np.float16).astype(np.float64)
                v32r = v32.reshape(NKB, 128, 64)
                wresr = wres.reshape(NKB, 128, 64)
                r = 1.0 / den[s * SEG:(s + 1) * SEG]
                rbar = r.reshape(4, 512).mean(axis=1)        # per chunk
                A = acs[:, s * 64:(s + 1) * 64].reshape(128, 4, NKB)
                Pp = acp[:, s * 64:(s + 1) * 64].reshape(128, 4, NKB)
                t1 = np.einsum('kcb,c->kb', A - Pp, rbar)
                t2 = np.einsum('kcb,c->kb', Pp, rbar)
                C += np.einsum('bkd,kb->d', v32r, t1)
                C += np.einsum('bkd,kb->d', wresr, t2)
            x[:, gcols] = x[:, gcols] / (3.0 * (Dv + C))[:, None]
        h0 = x[:, :4 * SEG]
        full[b, :, j, :] = h0.T
        h1 = x[:, 4 * SEG:6 * SEG]
        for g in range(2):
            full[b, g * 4096 + 1:(g + 1) * 4096:2, 4 + j, :] = \
                h1[:, g * SEG:(g + 1) * SEG].T
        full[b, 2::4, 8 + j, :] = x[:, 6 * SEG:].T
    return full


def _ensure_axon_backend():
    """The bass PJRT path needs the axon/neuron jax backend. A harness may
    pin JAX_PLATFORMS=cpu for its reference; re-select axon if so."""
    import jax
    try:
        plat = jax.devices()[0].platform
    except Exception:
        plat = ""
    if plat not in ("axon", "neuron"):
        try:
            jax.config.update("jax_platforms", "axon,cpu")
            jax.devices()
        except Exception:
            pass


def kernel(query, key, value):
    _ensure_axon_backend()
    query = np.asarray(query, np.float32)
    key = np.asarray(key, np.float32)
    value = np.asarray(value, np.float32)
    assert query.shape == (B, N, H, D)

    if "nc" not in _CACHE:
        _CACHE["nc"] = _build_nc()
    nc = _CACHE["nc"]

    in_maps = [_prep_core(query, key, value, c) for c in range(8)]
    res = run_bass_kernel_spmd(nc, in_maps, core_ids=list(range(8)))
    LAST_RESULT["exec_time_ns"] = res.exec_time_ns
    LAST_RESULT["results"] = res.results
    return _unshard(res.results, query, key, value, query.dtype)


# revision 6
# speedup vs baseline: 2.3247x; 2.3247x over previous
"""Dilated attention (LongNet-style) Trainium2 kernel, v3 (f32r-PV).

Sharding: 8 cores = 2 batches x 4 head columns; 7 dense 2048x2048x64
attention units per core (see v2 docstring).

Precision ("rowsum-corrected f32r attention"): the reference's
x / x.sum(axis=(1,2)) normalization amplifies per-element noise ~300x, but
ONLY through the denominator D = sum(x). Per-element noise in x itself is
unamplified. The PV matmul therefore runs with a SINGLE f32r stream:
  - P22 = round_f32r(64*exp(s)): ACT exp writes float32r directly (HW-
    probed: RNE to 11 mantissa bits). PE consumes it exactly (on-grid).
  - weights w22 = round_f32r(32*v | ones*32), pre-rounded ON HOST (same
    m11-RNE grid) so the residual wres = 32v - w22 is host-exact.
Host-side D repair (per dilation group, f64):
  D = sum_pos x  +  sum_chunks rbar_c * sum_k [ALPHA*32v + wres] * accS[k,c]
where accS[k,chunk] = sum_q P22 (DVE tensor_reduce export, f32) and ALPHA
is the mean relative rounding residual of the ACT f32->f32r conversion
(0 if RNE; ~2^-14.8 if truncation -- calibrated from the HW probe).
Scores DO feed the amplified path, so they keep the baseline's exact
high-precision path: two fp16 matmuls with shared [kh;kl] K=128-stacked
weights (khl @ qhh + khl @ qll, ~2^-22 score error). fp8 DoubleRow was
measured SLOWER than a second fp16 matmul here (DR streams ~1.13*N cycles
and re-loads 256 weight columns; the qll matmul reuses the loaded weights).

Device kernel per (chunk, k-block) unit (28 q-chunks of 512 x 16 k-blocks):
  S = khl_blk.T @ qhh + khl_blk.T @ qll  (fp16, 2x512cyc, 1 LDW) -> PSUM
  P22 = exp(S*ESC + ln64) -> SBUF f32r   (ACT, one batched pass per round)
  accS += rowsums(P22)                   (DVE, one batched reduce per round)
  O[65, 512] += w22_blk.T @ P22 (f32r, 512cyc; row 64 = denominator)
PE: 3 streams/unit = ~1536 cyc vs baseline's 5 = 2560. ACT ~1.6us/round
and DVE ~1.7us/round stay under the PE's ~1.9us so the PE never starves
(keeps the HAM clock-gate at 8/8 -- the v2 failure mode).
"""

import os
import sys

if "/opt/trn_rl_repo" not in sys.path:
    sys.path.insert(0, "/opt/trn_rl_repo")
if "jax" not in sys.modules:
    os.environ.setdefault("JAX_PLATFORMS", "axon")

import math

import ml_dtypes
import numpy as np

import concourse.bass as bass  # noqa: F401
import concourse.mybir as mybir
import concourse.tile as tile
from concourse import bacc
from concourse.bass_utils import run_bass_kernel_spmd

F32 = mybir.dt.float32
F32R = mybir.dt.float32r
F16 = mybir.dt.float16
F8 = mybir.dt.float8e4
NP_F8 = ml_dtypes.float8_e4m3

B, N, H, D = 2, 8192, 12, 64
NSEG = 7           # segments per core
SEG = 2048         # dilated segment length
NCHUNK = NSEG * 4  # 512-wide q chunks per core
NKB = 16           # 128-row k blocks per segment
NUNIT = NCHUNK * NKB
RW = 3             # units per round (3 PSUM banks per score tile)
QSC = np.float32(256.0)               # fp16 pre-scale for Q/K
VSC = np.float32(32.0)                # pre-scale for V (and ones col)
ESC = float(0.125 / (256.0 * 256.0))  # exp scale: 1/sqrt(64) + descale
PBIAS = float(math.log(64.0))         # exp bias: P in [0.3, 16K]

# ACT f32->f32r residual model: alpha = E[(P - f32r(P))/P]. HW-probed:
# every f32r producer (DMA, ACT out, DVE out, matmul read) rounds to
# nearest-even with 11 mantissa bits (e8m11), so the residual is zero-mean.
ALPHA = 0.0
M_BITS = 11        # f32r mantissa bits (for host weight pre-rounding)
RNE = True         # f32r rounding mode (for host weight pre-rounding)

_CACHE = {}
LAST_RESULT = {}


def _round_f32r(x, m=None, rne=None):
    m = M_BITS if m is None else m
    rne = RNE if rne is None else rne
    a = np.ascontiguousarray(x, np.float32)
    u = a.view(np.uint32).copy()
    drop = 23 - m
    if rne:
        u += np.uint32(1 << (drop - 1))
    u &= np.uint32((0xFFFFFFFF << drop) & 0xFFFFFFFF)
    return u.view(np.float32)


def _build_nc():
    nc = bacc.Bacc("TRN2", target_bir_lowering=False, debug=False,
                   enable_asserts=False, num_devices=8)
    qhh = nc.dram_tensor("qhh", [128, NSEG * SEG], F16, kind="ExternalInput")
    khl = nc.dram_tensor("khl", [128, NSEG * SEG], F16, kind="ExternalInput")
    qll = nc.dram_tensor("qll", [128, NSEG * SEG], F16, kind="ExternalInput")
    w22 = nc.dram_tensor("w22", [128, NSEG * NKB * 65], F32R,
                         kind="ExternalInput")
    out = nc.dram_tensor("out", [65, NCHUNK * 512], F32, kind="ExternalOutput")
    acs = nc.dram_tensor("acs", [128, NUNIT], F32, kind="ExternalOutput")
    qhh_ap, khl_ap, qll_ap, w22_ap, out_ap, acs_ap = (
        qhh.ap(), khl.ap(), qll.ap(), w22.ap(), out.ap(), acs.ap())

    with tile.TileContext(nc) as tc:
        with (
            tc.tile_pool(name="inp", bufs=1) as inp,
            tc.tile_pool(name="pt", bufs=4) as ptp,
            tc.tile_pool(name="osb", bufs=3) as osbp,
            tc.tile_pool(name="score", bufs=2, space="PSUM") as scp,
            tc.tile_pool(name="ot", bufs=2, space="PSUM") as otp,
        ):
            bias_t = inp.tile([128, 1], F32, tag="bias", name="bias_t")
            nc.vector.memset(bias_t[:, :], PBIAS)
            acs_sb = inp.tile([128, NUNIT], F32, tag="acs", name="acs_sb")

            # Warm-up prologue: runs while the input DMAs land. ~32 dummy
            # matmuls keep the PE busy >3.4us so the HAM clock-gate opens
            # before the real rounds, and one dummy exp pulls in the ACT
            # table load (~2.7us) that would otherwise stall round 0.
            wsrc = inp.tile([128, 128], F16, tag="wsrc", name="wsrc")
            wjunk = inp.tile([128, 512], F16, tag="wjunk", name="wjunk")
            nc.vector.memset(wsrc[:, :], 0.01)
            nc.vector.memset(wjunk[:, :], 0.01)
            warm = scp.tile([128, 512 * RW], F32, tag="score", name="warm")
            for i in range(14):
                nc.tensor.matmul(warm[:, (i % 3) * 512:(i % 3 + 1) * 512],
                                 wsrc[:, :], wjunk[:, :],
                                 start=(i < 3), stop=(i >= 11))
            wp = ptp.tile([128, 512 * RW], F32R, tag="p22", name="warmp")
            nc.scalar.activation(
                wp[:, :512], warm[:, :512],
                mybir.ActivationFunctionType.Exp, scale=ESC, bias=bias_t[:, :])

            qh_sb, ql_sb, k_sb, w_sb = [], [], [], []
            for s in range(NSEG):
                qh = inp.tile([128, SEG], F16, tag=f"qh{s}", name=f"qh{s}")
                ql = inp.tile([128, SEG], F16, tag=f"ql{s}", name=f"ql{s}")
                kk = inp.tile([128, SEG], F16, tag=f"k{s}", name=f"k{s}")
                wv = inp.tile([128, NKB * 65], F32R, tag=f"wv{s}",
                              name=f"wv{s}")
                # split the first segment's Q/K transfers across DMA queues so
                # round 0 isn't gated on a single ~512KB queue transfer
                nsl_dma = 4 if s == 0 else 1
                for t, ap_ in ((qh, qhh_ap), (ql, qll_ap), (kk, khl_ap)):
                    step = SEG // nsl_dma
                    for z in range(nsl_dma):
                        lo = z * step
                        nc.sync.dma_start(
                            t[:, lo:lo + step],
                            ap_[:, s * SEG + lo:s * SEG + lo + step])
                nc.sync.dma_start(
                    wv[:, :], w22_ap[:, s * NKB * 65:(s + 1) * NKB * 65])
                qh_sb.append(qh)
                ql_sb.append(ql)
                k_sb.append(kk)
                w_sb.append(wv)

            ot_tiles = {}
            pend1, pend2 = [], []  # PV work lagged by 1 and 2 rounds

            def flush(items):
                for p22ref, i, u in items:
                    cid, kb = divmod(u, NKB)
                    s = cid // 4
                    if kb == 0:
                        ot_tiles[cid] = otp.tile([65, 512], F32, tag="ot",
                                                 name=f"ot{cid}")
                    vsl = slice(kb * 65, (kb + 1) * 65)
                    psl = slice(i * 512, (i + 1) * 512)
                    nc.tensor.matmul(ot_tiles[cid][:, :], w_sb[s][:, vsl],
                                     p22ref[:, psl],
                                     start=(kb == 0), stop=(kb == NKB - 1))
                    if kb == NKB - 1:
                        o_sb = osbp.tile([65, 512], F32, tag="osb",
                                         name=f"osb{cid}")
                        # ACT does this copy: DVE is near its budget with the
                        # per-round reduce; ACT has ~400ns/round of slack.
                        nc.scalar.copy(o_sb[:, :], ot_tiles[cid][:, :])
                        nc.sync.dma_start(
                            out_ap[:, cid * 512:(cid + 1) * 512], o_sb[:, :])

            for r in range((NUNIT + RW - 1) // RW):
                units = range(r * RW, min((r + 1) * RW, NUNIT))
                nu = len(units)
                score = scp.tile([128, 512 * RW], F32, tag="score",
                                 name=f"score{r}")
                for i, u in enumerate(units):
                    cid, kb = divmod(u, NKB)
                    s, c = divmod(cid, 4)
                    osl = slice(i * 512, (i + 1) * 512)
                    csl = slice(c * 512, (c + 1) * 512)
                    ksl = slice(kb * 128, (kb + 1) * 128)
                    lhsT = k_sb[s][:, ksl]
                    nc.tensor.matmul(score[:, osl], lhsT, qh_sb[s][:, csl],
                                     start=True, stop=False)
                    nc.tensor.matmul(score[:, osl], lhsT, ql_sb[s][:, csl],
                                     start=False, stop=True)
                p22 = ptp.tile([128, 512 * RW], F32R, tag="p22",
                               name=f"p22_{r}")
                nsl = slice(0, 512 * nu)
                nc.scalar.activation(
                    p22[:, nsl], score[:, nsl],
                    mybir.ActivationFunctionType.Exp, scale=ESC,
                    bias=bias_t[:, :])
                u0 = r * RW
                nc.vector.tensor_reduce(
                    acs_sb[:, u0:u0 + nu],
                    p22[:, nsl].bitcast(F32).rearrange(
                        "p (u q) -> p u q", u=nu),
                    axis=mybir.AxisListType.X, op=mybir.AluOpType.add)
                if r < 2:
                    # startup filler: keep the PE streaming through the
                    # pipe-fill with dummies aimed at an OT-pool slot.
                    fill = otp.tile([128, 512], F32, tag="ot", name=f"fill{r}")
                    for z in range(3):
                        nc.tensor.matmul(fill[:, :], wsrc[:, :], wjunk[:, :],
                                         start=(z == 0), stop=(z == 2))
                flush(pend2)
                pend2 = pend1
                pend1 = [(p22, i, u) for i, u in enumerate(units)]
            flush(pend2)
            flush(pend1)
            nc.sync.dma_start(acs_ap[:, :], acs_sb[:, :])

    nc.compile()
    return nc


def _gather_segs(query, key, value, core):
    b, j = divmod(core, 4)
    segs = []
    for arr in (query, key, value):
        h0 = arr[b, :, j, :].reshape(4, SEG, D)
        h1 = arr[b, :, 4 + j, :].reshape(2, 4096, D)[:, 1::2, :]
        h2 = arr[b, 2::4, 8 + j, :][None]
        segs.append(np.concatenate([h0, h1, h2], axis=0))  # [7, 2048, 64]
    return segs


def _prep_core(query, key, value, core):
    qs, ks, vs = _gather_segs(query, key, value, core)
    # [64, NSEG*SEG] with col = s*SEG + p
    qt = (qs * QSC).transpose(2, 0, 1).reshape(D, NSEG * SEG)
    kt = (ks * QSC).transpose(2, 0, 1).reshape(D, NSEG * SEG)
    qh = qt.astype(np.float16)
    ql = (qt - qh).astype(np.float16)
    kh = kt.astype(np.float16)
    kl = (kt - kh).astype(np.float16)
    vv = np.concatenate(
        [vs * VSC, np.full((NSEG, SEG, 1), float(VSC), np.float32)],
        axis=2)  # [7, 2048, 65]
    w22 = _round_f32r(
        vv.reshape(NSEG, NKB, 128, 65).transpose(2, 0, 1, 3).reshape(128, -1))
    return {
        "qhh": np.ascontiguousarray(np.concatenate([qh, qh], axis=0)),
        "qll": np.ascontiguousarray(np.concatenate([ql, ql], axis=0)),
        "khl": np.ascontiguousarray(np.concatenate([kh, kl], axis=0)),
        "w22": np.ascontiguousarray(w22),
    }


def _unshard(results, query, key, value, dtype):
    full = np.zeros((B, N, H, D), dtype)
    groups = [(0, 4), (4, 6), (6, 7)]
    for core in range(8):
        b, j = divmod(core, 4)
        _, _, vs = _gather_segs(query, key, value, core)
        o = results[core]["out"].astype(np.float64)
        num, den = o[:64], o[64]                     # [64, 14336], [14336]
        acs = results[core]["acs"].astype(np.float64)  # [128, 448]
        x = num / den[None, :]
        for g0, g1 in groups:
            gcols = slice(g0 * SEG, g1 * SEG)
            Dv = x[:, gcols].sum(axis=1)             # [64]
            C = np.zeros(64)
            for s in range(g0, g1):
                v32 = (VSC * vs[s]).astype(np.float64)       # [2048, 64]
                w22v = _round_f32r(v32.astype(np.float32)).astype(np.float64)
                wres = v32 - w22v
                corr = (ALPHA * v32 + wres).reshape(NKB, 128, 64)
                r = 1.0 / den[s * SEG:(s + 1) * SEG]
                rbar = r.reshape(4, 512).mean(axis=1)        # per chunk
                A = acs[:, s * 64:(s + 1) * 64].reshape(128, 4, NKB)
                t1 = np.einsum('kcb,c->kb', A, rbar)
                C += np.einsum('bkd,kb->d', corr, t1)
            x[:, gcols] = x[:, gcols] / (3.0 * (Dv + C))[:, None]
        h0 = x[:, :4 * SEG]
        full[b, :, j, :] = h0.T
        h1 = x[:, 4 * SEG:6 * SEG]
        for g in range(2):
            full[b, g * 4096 + 1:(g + 1) * 4096:2, 4 + j, :] = \
                h1[:, g * SEG:(g + 1) * SEG].T
        full[b, 2::4, 8 + j, :] = x[:, 6 * SEG:].T
    return full


def _ensure_axon_backend():
    """The bass PJRT path needs the axon/neuron jax backend. A harness may
    pin JAX_PLATFORMS=cpu for its reference; re-select axon if so."""
    import jax
    try:
        plat = jax.devices()[0].platform
    except Exception:
        plat = ""
    if plat not in ("axon", "neuron"):
        try:
            jax.config.update("jax_platforms", "axon,cpu")
            jax.devices()
        except Exception:
            pass


def kernel(query, key, value):
    _ensure_axon_backend()
    query = np.asarray(query, np.float32)
    key = np.asarray(key, np.float32)
    value = np.asarray(value, np.float32)
    assert query.shape == (B, N, H, D)

    if "nc" not in _CACHE:
        _CACHE["nc"] = _build_nc()
    nc = _CACHE["nc"]

    in_maps = [_prep_core(query, key, value, c) for c in range(8)]
    res = run_bass_kernel_spmd(nc, in_maps, core_ids=list(range(8)))
    if not _consistent(res.results):
        # one-time transient flake seen on a first execution: accS and den
        # disagreed. Both sum the same P22 values, so a mismatch means a
        # corrupted run; retry once.
        res = run_bass_kernel_spmd(nc, in_maps, core_ids=list(range(8)))
    LAST_RESULT["exec_time_ns"] = res.exec_time_ns
    LAST_RESULT["results"] = res.results
    return _unshard(res.results, query, key, value, query.dtype)


def _consistent(results):
    for core in range(8):
        den = results[core]["out"][64].astype(np.float64)
        acs = results[core]["acs"].astype(np.float64)
        if not np.isfinite(den).all() or (den <= 0).any():
            return False
        # 32 * sum_k accS[:, chunk] == sum_{q in chunk} den_q
        lhs = 32.0 * acs.sum(axis=0).reshape(NCHUNK, NKB).sum(axis=1)
        rhs = den.reshape(NCHUNK, 512).sum(axis=1)
        if not np.allclose(lhs, rhs, rtol=1e-3):
            return False
    return True


# revision 7
# speedup vs baseline: 2.8605x; 1.2305x over previous
"""Dilated attention (LongNet-style) Trainium2 kernel, v4.

Problem: query/key/value (2, 8192, 12, 64) f32. Three dilation groups
(segment lengths 2048/4096/8192, dilation 1/2/4, head slices 0:4/4:8/8:12).
Each group's gather produces independent dense attention over 2048-position
dilated segments; outputs are normalized per (batch, head, channel) by the
sum over all segment positions, and divided by num_groups.

Sharding: 8 cores = 2 batches x 4 "head columns". Core c owns batch c//4 and
heads {j, 4+j, 8+j} where j = c%4 -- exactly 7 dense 2048x2048x64 attention
units per core (4 + 2 + 1 segments), perfectly balanced, with all segments of
any (batch, head) on one core so normalization needs no cross-core traffic.

Precision ("self-correcting f32r attention"): the reference's
x / x.sum(axis=(1,2)) normalization divides by a nearly-cancelling sum D,
which amplifies per-element noise ~300x -- but ONLY through D. Per-element
noise in x itself is unamplified, so the whole attention runs at reduced
precision and only D gets repaired:
  - P22 = f32r(64*exp(s)): ACT exp writes float32r (HW: round-to-nearest,
    11 mantissa bits). The PE consumes the same rounded values.
  - PV weights w22 = f32r(32*v), plus a denominator row of 32.0.
  - A matmul's cost is N cycles regardless of output partition count, so
    the 63 spare PSUM partitions of the PV matmul carry, FOR FREE, the
    w-rounding correction G[d] = sum_k (4096*(32v - w22))*P22 for channels
    0..62 (channel 63's correction is negligible in global L2 -- verified
    in simulation; 64+1+63 = 128 rows exactly fills the PSUM partition dim).
  - Host (f64): x = num/den;  D_d = sum_pos x_d + 2^-12 * sum_q r_q G[d,q];
    out = x / (3*D). The per-q r_q = 1/den_q makes G an essentially exact
    repair of the weight-rounding part of D's noise; the zero-mean P22
    rounding residual is left uncorrected (simulated total 2.9e-3 vs the
    2e-2 gate).
Scores feed the amplified path directly, so they keep k at ~fp32 via the
[kh;kl] K=128 stacking trick, with q at plain fp16 (the q-residual term was
simulated unnecessary): ONE fp16 matmul per unit. A ~160-col zero-weight
pad matmul keeps the PE stream denser than ACT's exp so the HAM clock-gate
stays at 8/8 (PE idling >~10% re-throttles the PE clock to 1.2GHz).

Device kernel per (chunk, k-block) unit (28 q-chunks of 512 x 16 k-blocks):
  S = khl_blk.T @ qhh (fp16, 512cyc) + 0-pad (160cyc) -> PSUM f32
  P22 = exp(S*ESC + ln64) -> SBUF f32r  (ACT, one batched pass per round)
  O[128, 512] += w22_blk.T @ P22 (f32r, 512cyc; rows: 64 num, 1 den, 63 G)
PE: ~2.3 streams/unit vs the 5-stream fp16-hi/lo baseline (510us). DVE only
copies chunk outputs. Engine budget per 3-unit round: PE ~1600ns (bottleneck
by design), ACT ~1490ns, DVE ~120ns.
"""

import os
import sys

if "/opt/trn_rl_repo" not in sys.path:
    sys.path.insert(0, "/opt/trn_rl_repo")
if "jax" not in sys.modules:
    os.environ.setdefault("JAX_PLATFORMS", "axon")

import math

import numpy as np

import concourse.bass as bass  # noqa: F401
import concourse.mybir as mybir
import concourse.tile as tile
from concourse import bacc
from concourse.bass_utils import run_bass_kernel_spmd

F32 = mybir.dt.float32
F32R = mybir.dt.float32r
F16 = mybir.dt.float16

B, N, H, D = 2, 8192, 12, 64
NSEG = 7           # segments per core
SEG = 2048         # dilated segment length
NCHUNK = NSEG * 4  # 512-wide q chunks per core
NKB = 16           # 128-row k blocks per segment
NUNIT = NCHUNK * NKB
RW = 3             # units per round (3 PSUM banks per score tile)
QSC = np.float32(256.0)               # fp16 pre-scale for Q/K
VSC = np.float32(32.0)                # pre-scale for V (and den row)
GSC = 4096.0                          # G-row scale: 2^12 * wres
ESC = float(0.125 / (256.0 * 256.0))  # exp scale: 1/sqrt(64) + descale
PBIAS = float(math.log(64.0))         # exp bias: P in [0.3, 16K]
PADN = 160         # zero-pad matmul columns (PE/ACT balance for HAM)

_CACHE = {}
LAST_RESULT = {}


def _round_f32r(x):
    """Round f32 to the HW f32r grid (round-to-nearest, 11 mantissa bits)."""
    a = np.ascontiguousarray(x, np.float32)
    u = a.view(np.uint32).copy()
    u += np.uint32(1 << 11)
    u &= np.uint32(0xFFFFF000)
    return u.view(np.float32)


def _build_nc():
    nc = bacc.Bacc("TRN2", target_bir_lowering=False, debug=False,
                   enable_asserts=False, num_devices=8)
    qhh = nc.dram_tensor("qhh", [128, NSEG * SEG], F16, kind="ExternalInput")
    khl = nc.dram_tensor("khl", [128, NSEG * SEG], F16, kind="ExternalInput")
    w22 = nc.dram_tensor("w22", [128, NSEG * NKB * 128], F32R,
                         kind="ExternalInput")
    out = nc.dram_tensor("out", [128, NCHUNK * 512], F32,
                         kind="ExternalOutput")
    qhh_ap, khl_ap, w22_ap, out_ap = qhh.ap(), khl.ap(), w22.ap(), out.ap()

    with tile.TileContext(nc) as tc:
        with (
            tc.tile_pool(name="inp", bufs=1) as inp,
            tc.tile_pool(name="pt", bufs=4) as ptp,
            tc.tile_pool(name="osb", bufs=3) as osbp,
            tc.tile_pool(name="score", bufs=2, space="PSUM") as scp,
            tc.tile_pool(name="ot", bufs=2, space="PSUM") as otp,
        ):
            bias_t = inp.tile([128, 1], F32, tag="bias", name="bias_t")
            nc.vector.memset(bias_t[:, :], PBIAS)
            zpad = inp.tile([128, 128], F16, tag="zpad", name="zpad")
            nc.vector.memset(zpad[:, :], 0.0)

            # Warm-up prologue: runs while the input DMAs land. Dummy matmuls
            # keep the PE busy >3.4us so the HAM clock-gate opens before the
            # real rounds, and one dummy exp pulls in the ACT table load
            # (~2.7us) that would otherwise stall round 0.
            wsrc = inp.tile([128, 128], F16, tag="wsrc", name="wsrc")
            wjunk = inp.tile([128, 512], F16, tag="wjunk", name="wjunk")
            nc.vector.memset(wsrc[:, :], 0.01)
            nc.vector.memset(wjunk[:, :], 0.01)
            warm = scp.tile([128, 512 * RW], F32, tag="score", name="warm")
            for i in range(14):
                nc.tensor.matmul(warm[:, (i % 3) * 512:(i % 3 + 1) * 512],
                                 wsrc[:, :], wjunk[:, :],
                                 start=(i < 3), stop=(i >= 11))
            wp = ptp.tile([128, 512 * RW], F32R, tag="p22", name="warmp")
            nc.scalar.activation(
                wp[:, :512], warm[:, :512],
                mybir.ActivationFunctionType.Exp, scale=ESC, bias=bias_t[:, :])

            qh_sb, k_sb, w_sb = [], [], []
            for s in range(NSEG):
                qh = inp.tile([128, SEG], F16, tag=f"qh{s}", name=f"qh{s}")
                kk = inp.tile([128, SEG], F16, tag=f"k{s}", name=f"k{s}")
                wv = inp.tile([128, NKB * 128], F32R, tag=f"wv{s}",
                              name=f"wv{s}")
                # split the first segment's Q/K transfers across DMA queues so
                # round 0 isn't gated on a single ~512KB queue transfer
                nsl_dma = 4 if s == 0 else 1
                for t, ap_ in ((qh, qhh_ap), (kk, khl_ap)):
                    step = SEG // nsl_dma
                    for z in range(nsl_dma):
                        lo = z * step
                        nc.sync.dma_start(
                            t[:, lo:lo + step],
                            ap_[:, s * SEG + lo:s * SEG + lo + step])
                nc.sync.dma_start(
                    wv[:, :], w22_ap[:, s * NKB * 128:(s + 1) * NKB * 128])
                qh_sb.append(qh)
                k_sb.append(kk)
                w_sb.append(wv)

            ot_tiles = {}
            pend1, pend2 = [], []  # PV work lagged by 1 and 2 rounds

            def flush(items):
                for p22ref, i, u in items:
                    cid, kb = divmod(u, NKB)
                    s = cid // 4
                    if kb == 0:
                        ot_tiles[cid] = otp.tile([128, 512], F32, tag="ot",
                                                 name=f"ot{cid}")
                    vsl = slice(kb * 128, (kb + 1) * 128)
                    psl = slice(i * 512, (i + 1) * 512)
                    nc.tensor.matmul(ot_tiles[cid][:, :], w_sb[s][:, vsl],
                                     p22ref[:, psl],
                                     start=(kb == 0), stop=(kb == NKB - 1))
                    if kb == NKB - 1:
                        o_sb = osbp.tile([128, 512], F32, tag="osb",
                                         name=f"osb{cid}")
                        nc.vector.tensor_copy(o_sb[:, :], ot_tiles[cid][:, :])
                        nc.sync.dma_start(
                            out_ap[:, cid * 512:(cid + 1) * 512], o_sb[:, :])

            for r in range((NUNIT + RW - 1) // RW):
                units = range(r * RW, min((r + 1) * RW, NUNIT))
                nu = len(units)
                score = scp.tile([128, 512 * RW], F32, tag="score",
                                 name=f"score{r}")
                for i, u in enumerate(units):
                    cid, kb = divmod(u, NKB)
                    s, c = divmod(cid, 4)
                    osl = slice(i * 512, (i + 1) * 512)
                    psl = slice(i * 512, i * 512 + PADN)
                    csl = slice(c * 512, (c + 1) * 512)
                    ksl = slice(kb * 128, (kb + 1) * 128)
                    nc.tensor.matmul(score[:, osl], k_sb[s][:, ksl],
                                     qh_sb[s][:, csl], start=True, stop=False)
                    # zero-weight pad: adds 0, keeps the PE stream above the
                    # ACT exp rate so the HAM clock-gate never re-throttles
                    nc.tensor.matmul(score[:, psl], zpad[:, :],
                                     qh_sb[s][:, c * 512:c * 512 + PADN],
                                     start=False, stop=True)
                p22 = ptp.tile([128, 512 * RW], F32R, tag="p22",
                               name=f"p22_{r}")
                nsl = slice(0, 512 * nu)
                nc.scalar.activation(
                    p22[:, nsl], score[:, nsl],
                    mybir.ActivationFunctionType.Exp, scale=ESC,
                    bias=bias_t[:, :])
                if r < 2:
                    # startup filler: keep the PE streaming through the
                    # pipe-fill with dummies aimed at an OT-pool slot.
                    fill = otp.tile([128, 512], F32, tag="ot", name=f"fill{r}")
                    for z in range(3):
                        nc.tensor.matmul(fill[:, :], wsrc[:, :], wjunk[:, :],
                                         start=(z == 0), stop=(z == 2))
                flush(pend2)
                pend2 = pend1
                pend1 = [(p22, i, u) for i, u in enumerate(units)]
            flush(pend2)
            flush(pend1)

    nc.compile()
    return nc


def _gather_segs(query, key, value, core):
    b, j = divmod(core, 4)
    segs = []
    for arr in (query, key, value):
        h0 = arr[b, :, j, :].reshape(4, SEG, D)
        h1 = arr[b, :, 4 + j, :].reshape(2, 4096, D)[:, 1::2, :]
        h2 = arr[b, 2::4, 8 + j, :][None]
        segs.append(np.concatenate([h0, h1, h2], axis=0))  # [7, 2048, 64]
    return segs


def _prep_core(query, key, value, core):
    qs, ks, vs = _gather_segs(query, key, value, core)
    # [64, NSEG*SEG] with col = s*SEG + p
    qt = (qs * QSC).transpose(2, 0, 1).reshape(D, NSEG * SEG)
    kt = (ks * QSC).transpose(2, 0, 1).reshape(D, NSEG * SEG)
    qh = qt.astype(np.float16)
    kh = kt.astype(np.float16)
    kl = (kt - kh).astype(np.float16)
    v32 = (vs * VSC).astype(np.float32)            # [7, 2048, 64]
    w22v = _round_f32r(v32)
    wres = _round_f32r(GSC * (v32.astype(np.float64)
                              - w22v.astype(np.float64)).astype(np.float32))
    # per (seg, kb) block [128 kpos, 128 outrows]:
    #   cols 0:64 = w22v, col 64 = 32.0, cols 65:128 = G weights (ch 0..62)
    wblk = np.empty((NSEG, SEG, 128), np.float32)
    wblk[:, :, :64] = w22v
    wblk[:, :, 64] = float(VSC)
    wblk[:, :, 65:] = wres[:, :, :63]
    w22 = (wblk.reshape(NSEG, NKB, 128, 128).transpose(2, 0, 1, 3)
           .reshape(128, -1))
    return {
        "qhh": np.ascontiguousarray(np.concatenate([qh, qh], axis=0)),
        "khl": np.ascontiguousarray(np.concatenate([kh, kl], axis=0)),
        "w22": np.ascontiguousarray(w22),
    }


def _unshard(results, dtype):
    full = np.zeros((B, N, H, D), dtype)
    groups = [(0, 4), (4, 6), (6, 7)]
    for core in range(8):
        b, j = divmod(core, 4)
        o = results[core]["out"].astype(np.float64)
        num, den, G = o[:64], o[64], o[65:]          # [64|63, 14336], [14336]
        r = 1.0 / den
        x = num * r[None, :]
        for g0, g1 in groups:
            gcols = slice(g0 * SEG, g1 * SEG)
            Dv = x[:, gcols].sum(axis=1)             # [64]
            C = np.zeros(64)
            C[:63] = (G[:, gcols] * r[None, gcols]).sum(axis=1) / GSC
            x[:, gcols] = x[:, gcols] / (3.0 * (Dv + C))[:, None]
        h0 = x[:, :4 * SEG]
        full[b, :, j, :] = h0.T
        h1 = x[:, 4 * SEG:6 * SEG]
        for g in range(2):
            full[b, g * 4096 + 1:(g + 1) * 4096:2, 4 + j, :] = \
                h1[:, g * SEG:(g + 1) * SEG].T
        full[b, 2::4, 8 + j, :] = x[:, 6 * SEG:].T
    return full


def _consistent(results):
    for core in range(8):
        den = results[core]["out"][64].astype(np.float64)
        if not np.isfinite(den).all() or (den <= 0).any():
            return False
        # den = 32 * sum_k P22 over 2048 rows, P22 in [0.3, 16K]:
        # sane bounds catch corrupted/partial runs
        if den.min() < 32 * 2048 * 0.01 or den.max() > 32 * 2048 * 2e4:
            return False
    return True


def _ensure_axon_backend():
    """The bass PJRT path needs the axon/neuron jax backend. A harness may
    pin JAX_PLATFORMS=cpu for its reference; re-select axon if so."""
    import jax
    try:
        plat = jax.devices()[0].platform
    except Exception:
        plat = ""
    if plat not in ("axon", "neuron"):
        try:
            jax.config.update("jax_platforms", "axon,cpu")
            jax.devices()
        except Exception:
            pass


def kernel(query, key, value):
    _ensure_axon_backend()
    query = np.asarray(query, np.float32)
    key = np.asarray(key, np.float32)
    value = np.asarray(value, np.float32)
    assert query.shape == (B, N, H, D)

    if "nc" not in _CACHE:
        _CACHE["nc"] = _build_nc()
    nc = _CACHE["nc"]

    in_maps = [_prep_core(query, key, value, c) for c in range(8)]
    res = run_bass_kernel_spmd(nc, in_maps, core_ids=list(range(8)))
    if not _consistent(res.results):
        # transient first-execution flakes have been observed once; both
        # checks can only fail on a corrupted run, so retry once.
        res = run_bass_kernel_spmd(nc, in_maps, core_ids=list(range(8)))
    LAST_RESULT["exec_time_ns"] = res.exec_time_ns
    LAST_RESULT["results"] = res.results
    return _unshard(res.results, query.dtype)


# revision 8
# speedup vs baseline: 2.9061x; 1.0160x over previous
"""Dilated attention (LongNet-style) Trainium2 kernel, v4.

Problem: query/key/value (2, 8192, 12, 64) f32. Three dilation groups
(segment lengths 2048/4096/8192, dilation 1/2/4, head slices 0:4/4:8/8:12).
Each group's gather produces independent dense attention over 2048-position
dilated segments; outputs are normalized per (batch, head, channel) by the
sum over all segment positions, and divided by num_groups.

Sharding: 8 cores = 2 batches x 4 "head columns". Core c owns batch c//4 and
heads {j, 4+j, 8+j} where j = c%4 -- exactly 7 dense 2048x2048x64 attention
units per core (4 + 2 + 1 segments), perfectly balanced, with all segments of
any (batch, head) on one core so normalization needs no cross-core traffic.

Precision ("self-correcting f32r attention"): the reference's
x / x.sum(axis=(1,2)) normalization divides by a nearly-cancelling sum D,
which amplifies per-element noise ~300x -- but ONLY through D. Per-element
noise in x itself is unamplified, so the whole attention runs at reduced
precision and only D gets repaired:
  - P22 = f32r(64*exp(s)): ACT exp writes float32r (HW: round-to-nearest,
    11 mantissa bits). The PE consumes the same rounded values.
  - PV weights w22 = f32r(32*v), plus a denominator row of 32.0.
  - A matmul's cost is N cycles regardless of output partition count, so
    the 63 spare PSUM partitions of the PV matmul carry, FOR FREE, the
    w-rounding correction G[d] = sum_k (4096*(32v - w22))*P22 for channels
    0..62 (channel 63's correction is negligible in global L2 -- verified
    in simulation; 64+1+63 = 128 rows exactly fills the PSUM partition dim).
  - Host (f64): x = num/den;  D_d = sum_pos x_d + 2^-12 * sum_q r_q G[d,q];
    out = x / (3*D). The per-q r_q = 1/den_q makes G an essentially exact
    repair of the weight-rounding part of D's noise; the zero-mean P22
    rounding residual is left uncorrected (simulated total 2.9e-3 vs the
    2e-2 gate).
Scores feed the amplified path directly, so they keep k at ~fp32 via the
[kh;kl] K=128 stacking trick, with q at plain fp16 (the q-residual term was
simulated unnecessary): ONE fp16 matmul per unit. A ~160-col zero-weight
pad matmul keeps the PE stream denser than ACT's exp so the HAM clock-gate
stays at 8/8 (PE idling >~10% re-throttles the PE clock to 1.2GHz).

Device kernel per (chunk, k-block) unit (28 q-chunks of 512 x 16 k-blocks):
  S = khl_blk.T @ qhh (fp16, 512cyc) + 0-pad (160cyc) -> PSUM f32
  P22 = exp(S*ESC + ln64) -> SBUF f32r  (ACT, one batched pass per round)
  O[128, 512] += w22_blk.T @ P22 (f32r, 512cyc; rows: 64 num, 1 den, 63 G)
PE: ~2.3 streams/unit vs the 5-stream fp16-hi/lo baseline (510us). DVE only
copies chunk outputs. Engine budget per 3-unit round: PE ~1600ns (bottleneck
by design), ACT ~1490ns, DVE ~120ns.
"""

import os
import sys

if "/opt/trn_rl_repo" not in sys.path:
    sys.path.insert(0, "/opt/trn_rl_repo")
if "jax" not in sys.modules:
    os.environ.setdefault("JAX_PLATFORMS", "axon")

import math

import numpy as np

import concourse.bass as bass  # noqa: F401
import concourse.mybir as mybir
import concourse.tile as tile
from concourse import bacc
from concourse.bass_utils import run_bass_kernel_spmd

F32 = mybir.dt.float32
F32R = mybir.dt.float32r
F16 = mybir.dt.float16

B, N, H, D = 2, 8192, 12, 64
NSEG = 7           # segments per core
SEG = 2048         # dilated segment length
NCHUNK = NSEG * 4  # 512-wide q chunks per core
NKB = 16           # 128-row k blocks per segment
NUNIT = NCHUNK * NKB
RW = 3             # units per round (3 PSUM banks per score tile)
QSC = np.float32(256.0)               # fp16 pre-scale for Q/K
VSC = np.float32(32.0)                # pre-scale for V (and den row)
GSC = 4096.0                          # G-row scale: 2^12 * wres
ESC = float(0.125 / (256.0 * 256.0))  # exp scale: 1/sqrt(64) + descale
PBIAS = float(math.log(64.0))         # exp bias: P in [0.3, 16K]
PADN = 128         # zero-pad matmul columns (PE/ACT balance for HAM)

_CACHE = {}
LAST_RESULT = {}


def _round_f32r(x):
    """Round f32 to the HW f32r grid (round-to-nearest, 11 mantissa bits)."""
    a = np.ascontiguousarray(x, np.float32)
    u = a.view(np.uint32).copy()
    u += np.uint32(1 << 11)
    u &= np.uint32(0xFFFFF000)
    return u.view(np.float32)


def _build_nc():
    nc = bacc.Bacc("TRN2", target_bir_lowering=False, debug=False,
                   enable_asserts=False, num_devices=8)
    qhh = nc.dram_tensor("qhh", [128, NSEG * SEG], F16, kind="ExternalInput")
    khl = nc.dram_tensor("khl", [128, NSEG * SEG], F16, kind="ExternalInput")
    w22 = nc.dram_tensor("w22", [128, NSEG * NKB * 128], F32R,
                         kind="ExternalInput")
    out = nc.dram_tensor("out", [128, NCHUNK * 512], F32,
                         kind="ExternalOutput")
    qhh_ap, khl_ap, w22_ap, out_ap = qhh.ap(), khl.ap(), w22.ap(), out.ap()

    with tile.TileContext(nc) as tc:
        with (
            tc.tile_pool(name="inp", bufs=1) as inp,
            tc.tile_pool(name="pt", bufs=4) as ptp,
            tc.tile_pool(name="osb", bufs=3) as osbp,
            tc.tile_pool(name="score", bufs=2, space="PSUM") as scp,
            tc.tile_pool(name="ot", bufs=2, space="PSUM") as otp,
        ):
            bias_t = inp.tile([128, 1], F32, tag="bias", name="bias_t")
            nc.vector.memset(bias_t[:, :], PBIAS)
            zpad = inp.tile([128, 128], F16, tag="zpad", name="zpad")
            nc.vector.memset(zpad[:, :], 0.0)

            # Warm-up prologue: runs while the input DMAs land. Dummy matmuls
            # keep the PE busy >3.4us so the HAM clock-gate opens before the
            # real rounds, and one dummy exp pulls in the ACT table load
            # (~2.7us) that would otherwise stall round 0.
            wsrc = inp.tile([128, 128], F16, tag="wsrc", name="wsrc")
            wjunk = inp.tile([128, 512], F16, tag="wjunk", name="wjunk")
            nc.vector.memset(wsrc[:, :], 0.01)
            nc.vector.memset(wjunk[:, :], 0.01)
            warm = scp.tile([128, 512 * RW], F32, tag="score", name="warm")
            for i in range(14):
                nc.tensor.matmul(warm[:, (i % 3) * 512:(i % 3 + 1) * 512],
                                 wsrc[:, :], wjunk[:, :],
                                 start=(i < 3), stop=(i >= 11))
            wp = ptp.tile([128, 512 * RW], F32R, tag="p22", name="warmp")
            nc.scalar.activation(
                wp[:, :512], warm[:, :512],
                mybir.ActivationFunctionType.Exp, scale=ESC, bias=bias_t[:, :])

            qh_sb, k_sb, w_sb = [], [], []
            for s in range(NSEG):
                qh = inp.tile([128, SEG], F16, tag=f"qh{s}", name=f"qh{s}")
                kk = inp.tile([128, SEG], F16, tag=f"k{s}", name=f"k{s}")
                wv = inp.tile([128, NKB * 128], F32R, tag=f"wv{s}",
                              name=f"wv{s}")
                # split the first segment's Q/K transfers across DMA queues so
                # round 0 isn't gated on a single ~512KB queue transfer
                nsl_dma = 4 if s == 0 else 1
                for t, ap_ in ((qh, qhh_ap), (kk, khl_ap)):
                    step = SEG // nsl_dma
                    for z in range(nsl_dma):
                        lo = z * step
                        nc.sync.dma_start(
                            t[:, lo:lo + step],
                            ap_[:, s * SEG + lo:s * SEG + lo + step])
                nc.sync.dma_start(
                    wv[:, :], w22_ap[:, s * NKB * 128:(s + 1) * NKB * 128])
                qh_sb.append(qh)
                k_sb.append(kk)
                w_sb.append(wv)

            ot_tiles = {}
            pend1, pend2 = [], []  # PV work lagged by 1 and 2 rounds

            def flush(items):
                for p22ref, i, u in items:
                    cid, kb = divmod(u, NKB)
                    s = cid // 4
                    if kb == 0:
                        ot_tiles[cid] = otp.tile([128, 512], F32, tag="ot",
                                                 name=f"ot{cid}")
                    vsl = slice(kb * 128, (kb + 1) * 128)
                    psl = slice(i * 512, (i + 1) * 512)
                    nc.tensor.matmul(ot_tiles[cid][:, :], w_sb[s][:, vsl],
                                     p22ref[:, psl],
                                     start=(kb == 0), stop=(kb == NKB - 1))
                    if kb == NKB - 1:
                        o_sb = osbp.tile([128, 512], F32, tag="osb",
                                         name=f"osb{cid}")
                        nc.vector.tensor_copy(o_sb[:, :], ot_tiles[cid][:, :])
                        nc.sync.dma_start(
                            out_ap[:, cid * 512:(cid + 1) * 512], o_sb[:, :])

            for r in range((NUNIT + RW - 1) // RW):
                units = range(r * RW, min((r + 1) * RW, NUNIT))
                nu = len(units)
                score = scp.tile([128, 512 * RW], F32, tag="score",
                                 name=f"score{r}")
                for i, u in enumerate(units):
                    cid, kb = divmod(u, NKB)
                    s, c = divmod(cid, 4)
                    osl = slice(i * 512, (i + 1) * 512)
                    psl = slice(i * 512, i * 512 + PADN)
                    csl = slice(c * 512, (c + 1) * 512)
                    ksl = slice(kb * 128, (kb + 1) * 128)
                    nc.tensor.matmul(score[:, osl], k_sb[s][:, ksl],
                                     qh_sb[s][:, csl], start=True, stop=False)
                    # zero-weight pad: adds 0, keeps the PE stream above the
                    # ACT exp rate so the HAM clock-gate never re-throttles
                    nc.tensor.matmul(score[:, psl], zpad[:, :],
                                     qh_sb[s][:, c * 512:c * 512 + PADN],
                                     start=False, stop=True)
                p22 = ptp.tile([128, 512 * RW], F32R, tag="p22",
                               name=f"p22_{r}")
                nsl = slice(0, 512 * nu)
                nc.scalar.activation(
                    p22[:, nsl], score[:, nsl],
                    mybir.ActivationFunctionType.Exp, scale=ESC,
                    bias=bias_t[:, :])
                if r < 2:
                    # startup filler: keep the PE streaming through the
                    # pipe-fill with dummies aimed at an OT-pool slot.
                    fill = otp.tile([128, 512], F32, tag="ot", name=f"fill{r}")
                    for z in range(5):
                        nc.tensor.matmul(fill[:, :], wsrc[:, :], wjunk[:, :],
                                         start=(z == 0), stop=(z == 4))
                flush(pend2)
                pend2 = pend1
                pend1 = [(p22, i, u) for i, u in enumerate(units)]
            flush(pend2)
            flush(pend1)

    nc.compile()
    return nc


def _gather_segs(query, key, value, core):
    b, j = divmod(core, 4)
    segs = []
    for arr in (query, key, value):
        h0 = arr[b, :, j, :].reshape(4, SEG, D)
        h1 = arr[b, :, 4 + j, :].reshape(2, 4096, D)[:, 1::2, :]
        h2 = arr[b, 2::4, 8 + j, :][None]
        segs.append(np.concatenate([h0, h1, h2], axis=0))  # [7, 2048, 64]
    return segs


def _prep_core(query, key, value, core):
    qs, ks, vs = _gather_segs(query, key, value, core)
    # [64, NSEG*SEG] with col = s*SEG + p
    qt = (qs * QSC).transpose(2, 0, 1).reshape(D, NSEG * SEG)
    kt = (ks * QSC).transpose(2, 0, 1).reshape(D, NSEG * SEG)
    qh = qt.astype(np.float16)
    kh = kt.astype(np.float16)
    kl = (kt - kh).astype(np.float16)
    v32 = (vs * VSC).astype(np.float32)            # [7, 2048, 64]
    w22v = _round_f32r(v32)
    wres = _round_f32r(GSC * (v32.astype(np.float64)
                              - w22v.astype(np.float64)).astype(np.float32))
    # per (seg, kb) block [128 kpos, 128 outrows]:
    #   cols 0:64 = w22v, col 64 = 32.0, cols 65:128 = G weights (ch 0..62)
    wblk = np.empty((NSEG, SEG, 128), np.float32)
    wblk[:, :, :64] = w22v
    wblk[:, :, 64] = float(VSC)
    wblk[:, :, 65:] = wres[:, :, :63]
    w22 = (wblk.reshape(NSEG, NKB, 128, 128).transpose(2, 0, 1, 3)
           .reshape(128, -1))
    return {
        "qhh": np.ascontiguousarray(np.concatenate([qh, qh], axis=0)),
        "khl": np.ascontiguousarray(np.concatenate([kh, kl], axis=0)),
        "w22": np.ascontiguousarray(w22),
    }


def _unshard(results, dtype):
    full = np.zeros((B, N, H, D), dtype)
    groups = [(0, 4), (4, 6), (6, 7)]
    for core in range(8):
        b, j = divmod(core, 4)
        o = results[core]["out"].astype(np.float64)
        num, den, G = o[:64], o[64], o[65:]          # [64|63, 14336], [14336]
        r = 1.0 / den
        x = num * r[None, :]
        for g0, g1 in groups:
            gcols = slice(g0 * SEG, g1 * SEG)
            Dv = x[:, gcols].sum(axis=1)             # [64]
            C = np.zeros(64)
            C[:63] = (G[:, gcols] * r[None, gcols]).sum(axis=1) / GSC
            x[:, gcols] = x[:, gcols] / (3.0 * (Dv + C))[:, None]
        h0 = x[:, :4 * SEG]
        full[b, :, j, :] = h0.T
        h1 = x[:, 4 * SEG:6 * SEG]
        for g in range(2):
            full[b, g * 4096 + 1:(g + 1) * 4096:2, 4 + j, :] = \
                h1[:, g * SEG:(g + 1) * SEG].T
        full[b, 2::4, 8 + j, :] = x[:, 6 * SEG:].T
    return full


def _consistent(results):
    for core in range(8):
        den = results[core]["out"][64].astype(np.float64)
        if not np.isfinite(den).all() or (den <= 0).any():
            return False
        # den = 32 * sum_k P22 over 2048 rows, P22 in [0.3, 16K]:
        # sane bounds catch corrupted/partial runs
        if den.min() < 32 * 2048 * 0.01 or den.max() > 32 * 2048 * 2e4:
            return False
    return True


def _ensure_axon_backend():
    """The bass PJRT path needs the axon/neuron jax backend. A harness may
    pin JAX_PLATFORMS=cpu for its reference; re-select axon if so."""
    import jax
    try:
        plat = jax.devices()[0].platform
    except Exception:
        plat = ""
    if plat not in ("axon", "neuron"):
        try:
            jax.config.update("jax_platforms", "axon,cpu")
            jax.devices()
        except Exception:
            pass


def kernel(query, key, value):
    _ensure_axon_backend()
    query = np.asarray(query, np.float32)
    key = np.asarray(key, np.float32)
    value = np.asarray(value, np.float32)
    assert query.shape == (B, N, H, D)

    if "nc" not in _CACHE:
        _CACHE["nc"] = _build_nc()
    nc = _CACHE["nc"]

    in_maps = [_prep_core(query, key, value, c) for c in range(8)]
    res = run_bass_kernel_spmd(nc, in_maps, core_ids=list(range(8)))
    if not _consistent(res.results):
        # transient first-execution flakes have been observed once; both
        # checks can only fail on a corrupted run, so retry once.
        res = run_bass_kernel_spmd(nc, in_maps, core_ids=list(range(8)))
    LAST_RESULT["exec_time_ns"] = res.exec_time_ns
    LAST_RESULT["results"] = res.results
    return _unshard(res.results, query.dtype)
